# revision 14
# baseline (speedup 1.0000x reference)
"""BitLinear kernel for Trainium2 (8 NeuronCores, tensor-parallel).

Computes: out = x @ (sign(w) * mean(|w|, axis=1, keepdims=True)).T
  x      : [4, 2048, 4096] f32
  weight : [4096, 4096] f32
  out    : [4, 2048, 4096] f32

Strategy: shard weight rows (out features) 8-way; each core computes a
[512, 8192] feature-major output shard.

Mixed-precision contraction (PE fill rate is the bottleneck): the first
16 k-tiles run as bf16 matmuls (~216ns per 128x512), the last 16 k-tiles
run as 8 fp8e4 DoubleRow pair-matmuls (2 k-tiles per instruction at
~232ns — 2x the contraction per instruction at ~+8% cost). sign(w) is
exact in both bf16 and fp8e4, so all quantization error comes from the
e4m3 x tiles: measured end-to-end rel-err 1.880e-2 (gate 2e-2). Host
computes signs and f32 scales; the device applies the per-feature scale
while evicting PSUM and stores the shard in bf16.

Layouts (host-pretiled so every DMA is a contiguous 256-512KB chunk
with 2KB-per-partition packets):
  xTp [8 pairs, 8, 128, 2, 1024] bf16 — two k-tiles interleaved per
      partition so one DMA loads both.
  xF  [8 pairs, 4, 128, 4096] fp8e4 — two DoubleRow pairs per chunk;
      per partition [jjA: ktA blk01 | ktB blk01, jjB: ...]; the matmul
      rhs AP is [128, 2 slots, 512] with slot stride 1024.
  sgB [128, 8192] bf16, sgF [128, 8192] fp8e4 — sign(w).T tiles,
      n-shard-major per k-tile; DR lhsT AP is [128, 2, 128], slot
      stride 512.
  sc  [128, 4] f32 — per-feature scales as n-tile columns.
  outT[512, 8192] bf16 — feature-major shard.

All tiles are allocated exactly once and ping-pong reused by parity
(the tile-framework teardown serializes per-allocation semaphore
cleanup — ~117 pool allocations cost ~8us of kernel tail).

8 warmup matmuls into the 8 distinct PSUM banks run during the
DMA-bound prologue with no data dependencies: they un-throttle the PE
HAM clock gate (cold 1.2GHz -> warm 2.4GHz needs ~3.4us of sustained
activity) before the first real matmul's operands land.
"""

import os
from contextlib import ExitStack

import numpy as np
import ml_dtypes

import concourse.bass as bass
import concourse.mybir as mybir
import concourse.tile as tile
from concourse import bacc, bass_utils

P = 128                 # SBUF partitions / PE array dim
D_IN = 4096             # contraction dim (in features)
D_OUT = 4096            # out features
M_TOT = 8192            # tokens (4*2048)
N_CORES = 8
N_SHARD = D_OUT // N_CORES      # 512 out features per core
K_TILES = D_IN // P             # 32
NB = 16                         # bf16 k-tiles (0..NB-1)
NFP = (K_TILES - NB) // 2       # 8 fp8 DoubleRow k-tile pairs (NB..31)
M_BLK = 512                     # moving free dim per matmul
M_BLKS = M_TOT // M_BLK         # 16
M_PAIRS = M_BLKS // 2           # 8 (x is loaded in block pairs)
N_TILES = N_SHARD // P          # 4
PAIR_W = 2 * M_BLK              # 1024

_CACHE = {}
LAST_RESULTS = None  # BassKernelResults of the most recent run (for test harness)


def _install_ntff_hook():
    """Register the ctypes NTFF profiling hook under antenv.axon_hooks so
    run_bass_kernel_spmd(trace=True) can capture device profiles under axon.
    No-op if already present or the .so lacks the symbols."""
    import contextlib
    import ctypes
    import sys
    import types

    try:
        from antenv.axon_hooks import get_axon_ntff_profile_hook  # noqa: F401

        return True
    except ImportError:
        pass

    so_path = "/opt/axon/libaxon_pjrt.so"
    if not os.path.exists(so_path):
        return False
    lib = ctypes.CDLL(so_path)
    if not hasattr(lib, "axon_start_nrt_profile"):
        return False
    lib.axon_start_nrt_profile.argtypes = [
        ctypes.POINTER(ctypes.c_int64),
        ctypes.c_size_t,
    ]
    lib.axon_start_nrt_profile.restype = ctypes.c_int64
    lib.axon_stop_nrt_profile.argtypes = [ctypes.c_char_p]
    lib.axon_stop_nrt_profile.restype = ctypes.c_int64

    @contextlib.contextmanager
    def _hook(output_dir, device_ids):
        import jax

        jax.devices()
        if device_ids:
            ids = (ctypes.c_int64 * len(device_ids))(*device_ids)
            rc = lib.axon_start_nrt_profile(ids, len(device_ids))
        else:
            rc = lib.axon_start_nrt_profile(None, 0)
        if rc != 0:
            raise RuntimeError(f"axon_start_nrt_profile rc={rc}")
        try:
            yield
        finally:
            n = lib.axon_stop_nrt_profile(str(output_dir).encode())
            print(f"ntff profile: {n} file(s) written to {output_dir}")

    mod = types.ModuleType("antenv.axon_hooks")
    _state = {"hook": _hook}
    mod.set_axon_ntff_profile_hook = lambda h: _state.__setitem__("hook", h)
    mod.get_axon_ntff_profile_hook = lambda: _state["hook"]
    sys.modules["antenv.axon_hooks"] = mod
    import antenv

    antenv.axon_hooks = mod

    # artifact upload reaches for a cloud bucket that isn't available here
    bass_utils.upload_artifacts = lambda tmpdir: f"local:{tmpdir}"
    return True


def _build_nc():
    nc = bacc.Bacc(
        "TRN2", target_bir_lowering=False, debug=False, num_devices=N_CORES,
        enable_partition_id=False,
    )
    xTp = nc.dram_tensor(
        "xTp", [M_PAIRS, NB, P, PAIR_W], mybir.dt.bfloat16, kind="ExternalInput"
    )
    xF = nc.dram_tensor(
        "xF", [M_PAIRS, NFP, P, 2 * PAIR_W], mybir.dt.float8e4,
        kind="ExternalInput",
    )
    sgB = nc.dram_tensor(
        "sgB", [P, NB * N_SHARD], mybir.dt.bfloat16, kind="ExternalInput"
    )
    sgF = nc.dram_tensor(
        "sgF", [P, NFP * 2 * N_SHARD], mybir.dt.float8e4, kind="ExternalInput"
    )
    sc = nc.dram_tensor("sc", [P, N_TILES], mybir.dt.float32, kind="ExternalInput")
    outT = nc.dram_tensor(
        "outT", [N_SHARD, M_TOT], mybir.dt.bfloat16, kind="ExternalOutput"
    )

    with tile.TileContext(nc) as tc, ExitStack() as ctx:
        sb = ctx.enter_context(tc.tile_pool(name="sb", bufs=1))
        pp = ctx.enter_context(tc.tile_pool(name="psum", bufs=1, space="PSUM"))

        # Every tile allocated exactly once; reuse is explicit by parity.
        sgB_t = sb.tile([P, NB, N_SHARD], mybir.dt.bfloat16)
        sgF_t = sb.tile([P, NFP, 2, N_SHARD], mybir.dt.float8e4)
        sct = sb.tile([P, N_TILES], mybir.dt.float32)
        warm = sb.tile([P, P + M_BLK], mybir.dt.bfloat16)
        xbs = [
            sb.tile([P, NB, PAIR_W], mybir.dt.bfloat16, name=f"xb{i}")
            for i in range(2)
        ]
        xfs = [
            sb.tile([P, NFP, 2, PAIR_W], mybir.dt.float8e4, name=f"xf{i}")
            for i in range(2)
        ]
        ops = [
            [
                sb.tile([P, PAIR_W], mybir.dt.bfloat16, name=f"op{i}_{ni}")
                for ni in range(N_TILES)
            ]
            for i in range(2)
        ]
        # 8 PSUM banks as two 4-bank sets; block g (= 2q+b) uses set g%2.
        psums = [
            [
                pp.tile([P, M_BLK], mybir.dt.float32, name=f"ps{i}_{ni}")
                for ni in range(N_TILES)
            ]
            for i in range(2)
        ]

        prev_sync_dma = [None]

        def sync_load(dst, src):
            dma = nc.sync.dma_start(dst, src)
            if prev_sync_dma[0] is not None:
                tile.add_dep_helper(
                    dma.ins, prev_sync_dma[0].ins, sync=False,
                    reason="sync DMA queue emission order",
                )
            prev_sync_dma[0] = dma
            return dma

        # ---- HAM warmup: a few dummy matmuls with no DMA dependencies give
        # the PE clock-gate monitor activity credit during the DMA-bound
        # prologue; sized to finish before the first real matmul's operands
        # land (~11us) so they never delay the real stream.
        nc.gpsimd.memset(warm[:], 0.0)
        for wi in range(4):
            nc.tensor.matmul(
                psums[0][wi][:], warm[:, 0:P], warm[:, P : P + M_BLK],
                start=True, stop=True,
            )

        def issue_x_pair(q):
            xb, xf = xbs[q % 2], xfs[q % 2]
            for j in range(NB):
                sync_load(xb[:, j, :], xTp[q, j, :, :])
            for jj in range(NFP):
                sync_load(xf[:, jj, :, :], xF[q, jj, :, :])
            return xb, xf

        def mm_b(pss, xb, b, ni, j):
            nc.tensor.matmul(
                pss[ni][:],
                sgB_t[:, j, ni * P : (ni + 1) * P],
                xb[:, j, b * M_BLK : (b + 1) * M_BLK],
                start=(j == 0),
                stop=False,
            )

        def mm_f(pss, xf, b, ni, jj):
            nc.tensor.matmul(
                pss[ni][:],
                sgF_t[:, jj, :, ni * P : (ni + 1) * P],
                xf[:, jj, :, b * M_BLK : (b + 1) * M_BLK],
                start=False,
                stop=(jj == NFP - 1),
                perf_mode=mybir.MatmulPerfMode.DoubleRow,
            )

        def evict_block(pss, opair, b):
            # Evictions alternate between the scalar and vector engines so
            # the per-block eviction chain (and the kernel tail) is half as
            # long.
            for ni in range(N_TILES):
                dst = opair[ni][:, b * M_BLK : (b + 1) * M_BLK]
                if ni % 2 == 0:
                    nc.scalar.mul(dst, pss[ni][:], sct[:, ni : ni + 1])
                else:
                    nc.vector.tensor_scalar_mul(dst, pss[ni][:], sct[:, ni : ni + 1])

        def store_pair(q, opair):
            for ni in range(N_TILES):
                dst = outT[ni * P : (ni + 1) * P, q * PAIR_W : (q + 1) * PAIR_W]
                nc.scalar.dma_start(dst, opair[ni][:])

        def store_half(q, opair, b, ni, queue):
            dst = outT[
                ni * P : (ni + 1) * P,
                q * PAIR_W + b * M_BLK : q * PAIR_W + (b + 1) * M_BLK,
            ]
            queue.dma_start(dst, opair[ni][:, b * M_BLK : (b + 1) * M_BLK])

        # ---- Prologue: chain sign chunks just ahead of the x tiles that
        # consume them so the earliest matmuls are fed with minimal latency.
        xb0, xf0 = xbs[0], xfs[0]
        # k-tile 0 signs ride the (idle) scalar queue in parallel with the
        # first x halves on the sync queue, so matmul #0's two operands
        # land concurrently instead of serially.
        nc.scalar.dma_start(sgB_t[:, 0, :], sgB[:, 0:N_SHARD])
        sync_load(xb0[:, 0, 0:M_BLK], xTp[0, 0, :, 0:M_BLK])
        sync_load(xb0[:, 0, M_BLK:PAIR_W], xTp[0, 0, :, M_BLK:])
        j = 1
        while j < NB:
            j2 = min(j + 2, NB)
            sync_load(sgB_t[:, j:j2, :], sgB[:, j * N_SHARD : j2 * N_SHARD])
            for jx in range(j, j2):
                sync_load(xb0[:, jx, :], xTp[0, jx, :, :])
            j = j2
        # fp8 signs in 2-pair chunks interleaved with pair-0 fp8 x
        for c in range(NFP // 2):
            sync_load(
                sgF_t[:, 2 * c : 2 * c + 2, :, :],
                sgF[:, c * 4 * N_SHARD : (c + 1) * 4 * N_SHARD],
            )
            sync_load(xf0[:, 2 * c, :, :], xF[0, 2 * c, :, :])
            sync_load(xf0[:, 2 * c + 1, :, :], xF[0, 2 * c + 1, :, :])
        # scales ride behind pair-0 (only needed at first eviction)
        sync_load(sct[:], sc[:, :])

        # ---- Main loop
        for q in range(M_PAIRS):
            xb, xf = (xb0, xf0) if q == 0 else issue_x_pair(q)
            opair = ops[q % 2]
            if q == 0:
                # j-outer across BOTH blocks (8 PSUM banks) so the PE keeps
                # pace with the HBM-limited startup stream.
                for j in range(NB):
                    for b in range(2):
                        for ni in range(N_TILES):
                            mm_b(psums[b], xb, b, ni, j)
                for jj in range(NFP):
                    for b in range(2):
                        for ni in range(N_TILES):
                            mm_f(psums[b], xf, b, ni, jj)
                for b in range(2):
                    evict_block(psums[b], opair, b)
                store_pair(q, opair)
            elif q < M_PAIRS - 1:
                for b in range(2):
                    pss = psums[b]
                    for j in range(NB):
                        for ni in range(N_TILES):
                            mm_b(pss, xb, b, ni, j)
                    for jj in range(NFP):
                        for ni in range(N_TILES):
                            mm_f(pss, xf, b, ni, jj)
                    evict_block(pss, opair, b)
                store_pair(q, opair)
            else:
                # Final pair: store each 512-block as soon as its eviction
                # completes (instead of waiting for the full 1024 pair tile)
                # and spread the tail stores across three DMA queues so the
                # kernel end isn't serialized on one ring.
                tailq = [nc.scalar, nc.gpsimd, nc.sync, nc.scalar]
                for b in range(2):
                    pss = psums[b]
                    if b == 0:
                        for j in range(NB):
                            for ni in range(N_TILES):
                                mm_b(pss, xb, b, ni, j)
                        for jj in range(NFP):
                            for ni in range(N_TILES):
                                mm_f(pss, xf, b, ni, jj)
                        evict_block(pss, opair, b)
                        for ni in range(N_TILES):
                            store_half(q, opair, b, ni, tailq[ni])
                    else:
                        # ni-outer: each n-tile's stop matmul lands early, so
                        # its eviction + store overlap the remaining matmuls
                        # instead of serializing after the last one.
                        for ni in range(N_TILES):
                            for j in range(NB):
                                mm_b(pss, xb, b, ni, j)
                            for jj in range(NFP):
                                mm_f(pss, xf, b, ni, jj)
                        for ni in range(N_TILES):
                            dst = opair[ni][:, M_BLK:PAIR_W]
                            if ni % 2 == 0:
                                nc.scalar.mul(dst, pss[ni][:], sct[:, ni : ni + 1])
                            else:
                                nc.vector.tensor_scalar_mul(
                                    dst, pss[ni][:], sct[:, ni : ni + 1]
                                )
                            store_half(q, opair, b, ni, tailq[ni])

    nc.compile()
    return nc


def kernel(x, weight):
    global LAST_RESULTS
    nc = _CACHE.get("nc")
    if nc is None:
        nc = _CACHE["nc"] = _build_nc()

    x = np.asarray(x)
    weight = np.asarray(weight)
    orig_shape = x.shape

    KB = NB * P  # contraction cols in bf16

    # Host-side layout: x.T pre-tiled; bf16 for k-tiles 0..NB-1, e4m3 for
    # the DoubleRow k-tile pairs.
    xT = x.reshape(M_TOT, D_IN).T  # [D_IN, M_TOT] view
    xTp = np.ascontiguousarray(
        xT[:KB].reshape(NB, P, M_PAIRS, PAIR_W)
        .transpose(2, 0, 1, 3)
        .astype(ml_dtypes.bfloat16)
    )  # [M_PAIRS, NB, P, 1024]
    # [NFP, 2t, P, M_PAIRS, 1024] -> [M_PAIRS, NFP, P, 2t, 1024]
    xF = np.ascontiguousarray(
        xT[KB:].reshape(NFP, 2, P, M_PAIRS, PAIR_W)
        .transpose(3, 0, 2, 1, 4)
        .reshape(M_PAIRS, NFP, P, 2 * PAIR_W)
        .astype(ml_dtypes.float8_e4m3fn)
    )

    SgT = np.sign(weight.T)  # [D_IN, D_OUT] f32, sign exact
    s_full = np.abs(weight.astype(np.float64)).mean(axis=1).astype(np.float32)

    in_maps = []
    for c in range(N_CORES):
        n0 = c * N_SHARD
        shard = SgT[:, n0 : n0 + N_SHARD]  # [D_IN, 512]
        # sgB[p, j*512+n] = sign(wT[j*128+p, n0+n])
        sgB = np.ascontiguousarray(
            shard[:KB].reshape(NB, P, N_SHARD)
            .transpose(1, 0, 2)
            .reshape(P, NB * N_SHARD)
            .astype(ml_dtypes.bfloat16)
        )
        # sgF[p, jj*1024 + t*512 + n] = sign(wT[(NB+2jj+t)*128+p, n0+n])
        sgF = np.ascontiguousarray(
            shard[KB:].reshape(NFP, 2, P, N_SHARD)
            .transpose(2, 0, 1, 3)
            .reshape(P, NFP * 2 * N_SHARD)
            .astype(ml_dtypes.float8_e4m3fn)
        )
        scl = np.ascontiguousarray(
            s_full[n0 : n0 + N_SHARD].reshape(N_TILES, P).T
        )  # [128, 4] f32
        in_maps.append({"xTp": xTp, "xF": xF, "sgB": sgB, "sgF": sgF, "sc": scl})

    trace = bool(int(os.environ.get("BITLIN_TRACE", "0")))
    if trace:
        trace = _install_ntff_hook()
        base = os.environ.get("BITLIN_TRACE_DIR") or None
        if base:
            import tempfile

            os.makedirs(base, exist_ok=True)
            tmpdir = tempfile.mkdtemp(dir=base)
        else:
            tmpdir = None
    else:
        tmpdir = None
    res = bass_utils.run_bass_kernel_spmd(
        nc, in_maps, core_ids=list(range(N_CORES)), trace=trace, tmpdir=tmpdir
    )
    LAST_RESULTS = res

    outT_full = np.concatenate(
        [np.asarray(res.results[c]["outT"]) for c in range(N_CORES)], axis=0
    )  # [D_OUT, M_TOT] bf16
    out = (
        np.ascontiguousarray(outT_full.T).astype(np.float32).reshape(orig_shape)
    )
    return out


# revision 16
# speedup vs baseline: 1.1929x; 1.1929x over previous
"""BitLinear kernel for Trainium2 (8 NeuronCores, tensor-parallel).

Computes: out = x @ (sign(w) * mean(|w|, axis=1, keepdims=True)).T
  x      : [4, 2048, 4096] f32
  weight : [4096, 4096] f32
  out    : [4, 2048, 4096] f32

Strategy: shard weight rows (out features) 8-way; each core computes a
[512, 8192] feature-major output shard.

Mixed-precision contraction (PE fill rate is the bottleneck): the first
16 k-tiles run as bf16 matmuls (~216ns per 128x512), the last 16 k-tiles
run as 8 fp8e4 DoubleRow pair-matmuls (2 k-tiles per instruction at
~232ns — 2x the contraction per instruction at ~+8% cost). sign(w) is
exact in both bf16 and fp8e4, so all quantization error comes from the
e4m3 x tiles: measured end-to-end rel-err 1.880e-2 (gate 2e-2). Host
computes signs and f32 scales; the device applies the per-feature scale
while evicting PSUM and stores the shard in bf16.

Layouts (host-pretiled so every DMA is a contiguous 256KB chunk with
2KB-per-partition packets; finer chunks would run the DMA queue slower,
coarser ones starve the HBM-paced startup feed):
  xTp [8 pairs, 16, 128, 1024] bf16 — x.T k-tile chunks per 1024-token
      pair (two 512 blocks).
  xF  [8 pairs, 8, 128, 2048] fp8e4 — DoubleRow pairs: per partition
      [ktA blk0 | ktA blk1 | ktB blk0 | ktB blk1]; the matmul rhs AP
      is [128, 2 slots, 512] with slot stride 1024.
  sgB [128, 8192] bf16, sgF [128, 8192] fp8e4 — sign(w).T tiles,
      n-shard-major per k-tile; DR lhsT AP is [128, 2, 128], slot
      stride 512.
  sc  [128, 4] f32 — per-feature scales as n-tile columns.
  outT[512, 8192] bf16 — feature-major shard.

All tiles are allocated exactly once and ping-pong reused by parity.
4 warmup matmuls with no data dependencies run during the DMA-bound
prologue, sized to finish as the first real operands land: they give
the PE HAM clock gate (cold 1.2GHz -> warm 2.4GHz after ~3.4us of
sustained activity) a head start without delaying the real stream.

Measured: ~349.5us HW exec (at the 2.4GHz power state; the chip's P0
downclock to 2.0GHz under sustained load adds ~20%), rel-err 1.880e-2
vs the f32 reference — the matmul stream is gapless at the PE fill
rate, bounded below by ~7us framework preamble, ~3us first-DMA
latency, and ~11.5us fixed teardown tail.
"""

import os
from contextlib import ExitStack

import numpy as np
import ml_dtypes

import concourse.bass as bass
import concourse.mybir as mybir
import concourse.tile as tile
from concourse import bacc, bass_utils

P = 128                 # SBUF partitions / PE array dim
D_IN = 4096             # contraction dim (in features)
D_OUT = 4096            # out features
M_TOT = 8192            # tokens (4*2048)
N_CORES = 8
N_SHARD = D_OUT // N_CORES      # 512 out features per core
K_TILES = D_IN // P             # 32
NB = 16                         # bf16 k-tiles (0..NB-1)
NFP = (K_TILES - NB) // 2       # 8 fp8 DoubleRow k-tile pairs (NB..31)
M_BLK = 512                     # moving free dim per matmul
M_BLKS = M_TOT // M_BLK         # 16
M_PAIRS = M_BLKS // 2           # 8 (x is loaded in block pairs)
N_TILES = N_SHARD // P          # 4
PAIR_W = 2 * M_BLK              # 1024

_CACHE = {}
LAST_RESULTS = None  # BassKernelResults of the most recent run (for test harness)


def _install_ntff_hook():
    """Register the ctypes NTFF profiling hook under antenv.axon_hooks so
    run_bass_kernel_spmd(trace=True) can capture device profiles under axon.
    No-op if already present or the .so lacks the symbols."""
    import contextlib
    import ctypes
    import sys
    import types

    try:
        from antenv.axon_hooks import get_axon_ntff_profile_hook  # noqa: F401

        return True
    except ImportError:
        pass

    so_path = "/opt/axon/libaxon_pjrt.so"
    if not os.path.exists(so_path):
        return False
    lib = ctypes.CDLL(so_path)
    if not hasattr(lib, "axon_start_nrt_profile"):
        return False
    lib.axon_start_nrt_profile.argtypes = [
        ctypes.POINTER(ctypes.c_int64),
        ctypes.c_size_t,
    ]
    lib.axon_start_nrt_profile.restype = ctypes.c_int64
    lib.axon_stop_nrt_profile.argtypes = [ctypes.c_char_p]
    lib.axon_stop_nrt_profile.restype = ctypes.c_int64

    @contextlib.contextmanager
    def _hook(output_dir, device_ids):
        import jax

        jax.devices()
        if device_ids:
            ids = (ctypes.c_int64 * len(device_ids))(*device_ids)
            rc = lib.axon_start_nrt_profile(ids, len(device_ids))
        else:
            rc = lib.axon_start_nrt_profile(None, 0)
        if rc != 0:
            raise RuntimeError(f"axon_start_nrt_profile rc={rc}")
        try:
            yield
        finally:
            n = lib.axon_stop_nrt_profile(str(output_dir).encode())
            print(f"ntff profile: {n} file(s) written to {output_dir}")

    mod = types.ModuleType("antenv.axon_hooks")
    _state = {"hook": _hook}
    mod.set_axon_ntff_profile_hook = lambda h: _state.__setitem__("hook", h)
    mod.get_axon_ntff_profile_hook = lambda: _state["hook"]
    sys.modules["antenv.axon_hooks"] = mod
    import antenv

    antenv.axon_hooks = mod

    # artifact upload reaches for a cloud bucket that isn't available here
    bass_utils.upload_artifacts = lambda tmpdir: f"local:{tmpdir}"
    return True


def _build_nc():
    nc = bacc.Bacc(
        "TRN2", target_bir_lowering=False, debug=False, num_devices=N_CORES,
        enable_partition_id=False,
    )
    xTp = nc.dram_tensor(
        "xTp", [M_PAIRS, NB, P, PAIR_W], mybir.dt.bfloat16, kind="ExternalInput"
    )
    xF = nc.dram_tensor(
        "xF", [M_PAIRS, NFP, P, 2 * PAIR_W], mybir.dt.float8e4,
        kind="ExternalInput",
    )
    sgB = nc.dram_tensor(
        "sgB", [P, NB * N_SHARD], mybir.dt.bfloat16, kind="ExternalInput"
    )
    sgF = nc.dram_tensor(
        "sgF", [P, NFP * 2 * N_SHARD], mybir.dt.float8e4, kind="ExternalInput"
    )
    sc = nc.dram_tensor("sc", [P, N_TILES], mybir.dt.float32, kind="ExternalInput")
    outT = nc.dram_tensor(
        "outT", [N_SHARD, M_TOT], mybir.dt.bfloat16, kind="ExternalOutput"
    )

    with tile.TileContext(nc) as tc, ExitStack() as ctx:
        sb = ctx.enter_context(tc.tile_pool(name="sb", bufs=1))
        pp = ctx.enter_context(tc.tile_pool(name="psum", bufs=1, space="PSUM"))

        # Every tile allocated exactly once; reuse is explicit by parity.
        sgB_t = sb.tile([P, NB, N_SHARD], mybir.dt.bfloat16)
        sgF_t = sb.tile([P, NFP, 2, N_SHARD], mybir.dt.float8e4)
        sct = sb.tile([P, N_TILES], mybir.dt.float32)
        warm = sb.tile([P, P + M_BLK], mybir.dt.bfloat16)
        xbs = [
            sb.tile([P, NB, PAIR_W], mybir.dt.bfloat16, name=f"xb{i}")
            for i in range(2)
        ]
        xfs = [
            sb.tile([P, NFP, 2, PAIR_W], mybir.dt.float8e4, name=f"xf{i}")
            for i in range(2)
        ]
        ops = [
            [
                sb.tile([P, PAIR_W], mybir.dt.bfloat16, name=f"op{i}_{ni}")
                for ni in range(N_TILES)
            ]
            for i in range(2)
        ]
        # 8 PSUM banks as two 4-bank sets; block g (= 2q+b) uses set g%2.
        psums = [
            [
                pp.tile([P, M_BLK], mybir.dt.float32, name=f"ps{i}_{ni}")
                for ni in range(N_TILES)
            ]
            for i in range(2)
        ]

        prev_sync_dma = [None]

        def sync_load(dst, src):
            dma = nc.sync.dma_start(dst, src)
            if prev_sync_dma[0] is not None:
                tile.add_dep_helper(
                    dma.ins, prev_sync_dma[0].ins, sync=False,
                    reason="sync DMA queue emission order",
                )
            prev_sync_dma[0] = dma
            return dma

        # ---- HAM warmup: a few dummy matmuls with no DMA dependencies give
        # the PE clock-gate monitor activity credit during the DMA-bound
        # prologue; sized to finish before the first real matmul's operands
        # land (~11us) so they never delay the real stream.
        nc.gpsimd.memset(warm[:], 0.0)
        for wi in range(4):
            nc.tensor.matmul(
                psums[0][wi][:], warm[:, 0:P], warm[:, P : P + M_BLK],
                start=True, stop=True,
            )

        def issue_x_pair(q):
            xb, xf = xbs[q % 2], xfs[q % 2]
            for j in range(NB):
                sync_load(xb[:, j, :], xTp[q, j, :, :])
            for jj in range(NFP):
                sync_load(xf[:, jj, :, :], xF[q, jj, :, :])
            return xb, xf

        def mm_b(pss, xb, b, ni, j):
            nc.tensor.matmul(
                pss[ni][:],
                sgB_t[:, j, ni * P : (ni + 1) * P],
                xb[:, j, b * M_BLK : (b + 1) * M_BLK],
                start=(j == 0),
                stop=False,
            )

        def mm_f(pss, xf, b, ni, jj):
            nc.tensor.matmul(
                pss[ni][:],
                sgF_t[:, jj, :, ni * P : (ni + 1) * P],
                xf[:, jj, :, b * M_BLK : (b + 1) * M_BLK],
                start=False,
                stop=(jj == NFP - 1),
                perf_mode=mybir.MatmulPerfMode.DoubleRow,
            )

        def evict_block(pss, opair, b):
            # Evictions alternate between the scalar and vector engines so
            # the per-block eviction chain (and the kernel tail) is half as
            # long.
            for ni in range(N_TILES):
                dst = opair[ni][:, b * M_BLK : (b + 1) * M_BLK]
                if ni % 2 == 0:
                    nc.scalar.mul(dst, pss[ni][:], sct[:, ni : ni + 1])
                else:
                    nc.vector.tensor_scalar_mul(dst, pss[ni][:], sct[:, ni : ni + 1])

        def store_pair(q, opair):
            for ni in range(N_TILES):
                dst = outT[ni * P : (ni + 1) * P, q * PAIR_W : (q + 1) * PAIR_W]
                nc.scalar.dma_start(dst, opair[ni][:])

        def store_half(q, opair, b, ni, queue):
            dst = outT[
                ni * P : (ni + 1) * P,
                q * PAIR_W + b * M_BLK : q * PAIR_W + (b + 1) * M_BLK,
            ]
            queue.dma_start(dst, opair[ni][:, b * M_BLK : (b + 1) * M_BLK])

        # ---- Prologue: chain sign chunks just ahead of the x tiles that
        # consume them so the earliest matmuls are fed with minimal latency.
        xb0, xf0 = xbs[0], xfs[0]
        # k-tile 0 signs ride the (idle) scalar queue in parallel with the
        # first x halves on the sync queue, so matmul #0's two operands
        # land concurrently instead of serially.
        nc.scalar.dma_start(sgB_t[:, 0, :], sgB[:, 0:N_SHARD])
        sync_load(xb0[:, 0, 0:M_BLK], xTp[0, 0, :, 0:M_BLK])
        sync_load(xb0[:, 0, M_BLK:PAIR_W], xTp[0, 0, :, M_BLK:])
        j = 1
        while j < NB:
            j2 = min(j + 2, NB)
            sync_load(sgB_t[:, j:j2, :], sgB[:, j * N_SHARD : j2 * N_SHARD])
            for jx in range(j, j2):
                sync_load(xb0[:, jx, :], xTp[0, jx, :, :])
            j = j2
        # fp8 signs in 2-pair chunks interleaved with pair-0 fp8 x
        for c in range(NFP // 2):
            sync_load(
                sgF_t[:, 2 * c : 2 * c + 2, :, :],
                sgF[:, c * 4 * N_SHARD : (c + 1) * 4 * N_SHARD],
            )
            sync_load(xf0[:, 2 * c, :, :], xF[0, 2 * c, :, :])
            sync_load(xf0[:, 2 * c + 1, :, :], xF[0, 2 * c + 1, :, :])
        # scales ride behind pair-0 (only needed at first eviction)
        sync_load(sct[:], sc[:, :])

        # ---- Main loop
        for q in range(M_PAIRS):
            xb, xf = (xb0, xf0) if q == 0 else issue_x_pair(q)
            opair = ops[q % 2]
            if q == 0:
                # j-outer across BOTH blocks (8 PSUM banks) so the PE keeps
                # pace with the HBM-limited startup stream.
                for j in range(NB):
                    for b in range(2):
                        for ni in range(N_TILES):
                            mm_b(psums[b], xb, b, ni, j)
                for jj in range(NFP):
                    for b in range(2):
                        for ni in range(N_TILES):
                            mm_f(psums[b], xf, b, ni, jj)
                for b in range(2):
                    evict_block(psums[b], opair, b)
                store_pair(q, opair)
            elif q < M_PAIRS - 1:
                for b in range(2):
                    pss = psums[b]
                    for j in range(NB):
                        for ni in range(N_TILES):
                            mm_b(pss, xb, b, ni, j)
                    for jj in range(NFP):
                        for ni in range(N_TILES):
                            mm_f(pss, xf, b, ni, jj)
                    evict_block(pss, opair, b)
                store_pair(q, opair)
            else:
                # Final pair: store each 512-block as soon as its eviction
                # completes (instead of waiting for the full 1024 pair tile)
                # and spread the tail stores across three DMA queues so the
                # kernel end isn't serialized on one ring.
                tailq = [nc.scalar, nc.gpsimd, nc.sync, nc.scalar]
                for b in range(2):
                    pss = psums[b]
                    if b == 0:
                        for j in range(NB):
                            for ni in range(N_TILES):
                                mm_b(pss, xb, b, ni, j)
                        for jj in range(NFP):
                            for ni in range(N_TILES):
                                mm_f(pss, xf, b, ni, jj)
                        evict_block(pss, opair, b)
                        for ni in range(N_TILES):
                            store_half(q, opair, b, ni, tailq[ni])
                    else:
                        # ni-outer: each n-tile's stop matmul lands early, so
                        # its eviction + store overlap the remaining matmuls
                        # instead of serializing after the last one.
                        for ni in range(N_TILES):
                            for j in range(NB):
                                mm_b(pss, xb, b, ni, j)
                            for jj in range(NFP):
                                mm_f(pss, xf, b, ni, jj)
                        for ni in range(N_TILES):
                            dst = opair[ni][:, M_BLK:PAIR_W]
                            if ni % 2 == 0:
                                nc.scalar.mul(dst, pss[ni][:], sct[:, ni : ni + 1])
                            else:
                                nc.vector.tensor_scalar_mul(
                                    dst, pss[ni][:], sct[:, ni : ni + 1]
                                )
                            store_half(q, opair, b, ni, tailq[ni])

    nc.compile()
    return nc


def kernel(x, weight):
    global LAST_RESULTS
    nc = _CACHE.get("nc")
    if nc is None:
        nc = _CACHE["nc"] = _build_nc()

    x = np.asarray(x)
    weight = np.asarray(weight)
    orig_shape = x.shape

    KB = NB * P  # contraction cols in bf16

    # Host-side layout: x.T pre-tiled; bf16 for k-tiles 0..NB-1, e4m3 for
    # the DoubleRow k-tile pairs.
    xT = x.reshape(M_TOT, D_IN).T  # [D_IN, M_TOT] view
    xTp = np.ascontiguousarray(
        xT[:KB].reshape(NB, P, M_PAIRS, PAIR_W)
        .transpose(2, 0, 1, 3)
        .astype(ml_dtypes.bfloat16)
    )  # [M_PAIRS, NB, P, 1024]
    # [NFP, 2t, P, M_PAIRS, 1024] -> [M_PAIRS, NFP, P, 2t, 1024]
    xF = np.ascontiguousarray(
        xT[KB:].reshape(NFP, 2, P, M_PAIRS, PAIR_W)
        .transpose(3, 0, 2, 1, 4)
        .reshape(M_PAIRS, NFP, P, 2 * PAIR_W)
        .astype(ml_dtypes.float8_e4m3fn)
    )

    SgT = np.sign(weight.T)  # [D_IN, D_OUT] f32, sign exact
    s_full = np.abs(weight.astype(np.float64)).mean(axis=1).astype(np.float32)

    in_maps = []
    for c in range(N_CORES):
        n0 = c * N_SHARD
        shard = SgT[:, n0 : n0 + N_SHARD]  # [D_IN, 512]
        # sgB[p, j*512+n] = sign(wT[j*128+p, n0+n])
        sgB = np.ascontiguousarray(
            shard[:KB].reshape(NB, P, N_SHARD)
            .transpose(1, 0, 2)
            .reshape(P, NB * N_SHARD)
            .astype(ml_dtypes.bfloat16)
        )
        # sgF[p, jj*1024 + t*512 + n] = sign(wT[(NB+2jj+t)*128+p, n0+n])
        sgF = np.ascontiguousarray(
            shard[KB:].reshape(NFP, 2, P, N_SHARD)
            .transpose(2, 0, 1, 3)
            .reshape(P, NFP * 2 * N_SHARD)
            .astype(ml_dtypes.float8_e4m3fn)
        )
        scl = np.ascontiguousarray(
            s_full[n0 : n0 + N_SHARD].reshape(N_TILES, P).T
        )  # [128, 4] f32
        in_maps.append({"xTp": xTp, "xF": xF, "sgB": sgB, "sgF": sgF, "sc": scl})

    trace = bool(int(os.environ.get("BITLIN_TRACE", "0")))
    if trace:
        trace = _install_ntff_hook()
        base = os.environ.get("BITLIN_TRACE_DIR") or None
        if base:
            import tempfile

            os.makedirs(base, exist_ok=True)
            tmpdir = tempfile.mkdtemp(dir=base)
        else:
            tmpdir = None
    else:
        tmpdir = None
    res = bass_utils.run_bass_kernel_spmd(
        nc, in_maps, core_ids=list(range(N_CORES)), trace=trace, tmpdir=tmpdir
    )
    LAST_RESULTS = res

    outT_full = np.concatenate(
        [np.asarray(res.results[c]["outT"]) for c in range(N_CORES)], axis=0
    )  # [D_OUT, M_TOT] bf16
    out = (
        np.ascontiguousarray(outT_full.T).astype(np.float32).reshape(orig_shape)
    )
    return out
